# revision 50
# baseline (speedup 1.0000x reference)
"""Trainium2 Bass kernel for nn_CrossAttention_47502338294587.

Math: the reference cross-attention has a single KV position broadcast over
all T query positions.  Softmax over a row of identical logits is uniform,
so attention output == v for every query, and the whole module collapses to

    out[b, t, :] = (visual_features[b] @ Wv + bv) @ Wp + bp      (for all t)

independent of x / Wq / Wk.  The device computes the two projections and
broadcasts the per-batch row over the T axis; the host only does input
layout prep (incl. bf16 weight packing) and shard re-assembly.

Sharding: tensor-parallel over the output channel dim C - core i computes
and writes out[:, :, i*128:(i+1)*128] (full Wv, column shard of Wp / bp).

Pipeline (per core):
  Wv streams as 8 column blocks wv_j = Wv[:, j*128:(j+1)*128] packed
  [p, k, c] = Wv[k*128+p, j*128+c], 256KB each, balanced across both
  HWDGE queues (12 input DMAs; the 4 tiny const transfers complete
  early so DMA-sem reuse never stalls an issue).  As each block lands
  the PE computes the transposed projection directly:

    psum_vvT_j[c', b] = bv_j (K=1 opener) + sum_k wv_j[:,k,:]^T @ vfT_k

  (no 4-partition PSUM casts, no PE transposes).  A [128,4] DVE copy
  drops vvT_j to SBUF and one accumulating matmul folds it into the
  output row while later blocks still stream (variant "prow", default):

    psum_row[b, ci] = bp (K=1 opener) + sum_j vvT_j^T @ Wp_j

  The broadcast uses a constant one-hot mask (no DVE multiply):

    pbcT[ci, (tc, b')] = sum_b prow[b, ci] * mask[b, (tc, b')],
    mask[b, (tc,b')] = (b == b')

  Variant "fold" instead folds each block straight into the broadcast
  PSUM tile with a tc-stride-0 replicated rhs (no prow/mask stage);
  "prow1" is prow with a single-queue out DMA.  All three are within
  run-to-run noise of each other; prow matched the best measurements.

  The resulting [128, 512] bf16 tile is replicated by the out DMA over
  the 8 T-chunks, split across both HWDGE queues.  Out layout is
  out[p=ci_local, q, tc, b] (1KB contiguous runs); the host transposes
  and upcasts to f32 during re-assembly.
"""

import os
import sys

import numpy as np

for _p in ("/opt/trn_rl_repo",):
    if _p not in sys.path and os.path.isdir(_p):
        sys.path.insert(0, _p)

B, T, C = 4, 1024, 1024
N_CORES = 8
CSH = C // N_CORES  # 128, C-shard per core
KC = C // 128  # 8 contraction chunks

_BUILT = {}
VARIANT = os.environ.get("KERNEL_VARIANT", "prow")


def build_nc(variant="fold"):
    """Build + compile the Bass program (one NeuronCore's SPMD body).

    variant "fold": per-block matmul folds directly into the broadcast
    PSUM tile via a tc-replicated rhs (short tail).
    variant "prow": per-block matmul accumulates prow[b, ci] (N=128),
    then a one-hot-mask matmul broadcasts it (v2/v4 tail).
    variant "prow1": prow tail with a single-queue out DMA.
    """
    import concourse.bass as bass
    import concourse.mybir as mybir
    import concourse.tile as tile
    from concourse import bacc

    f32 = mybir.dt.float32
    bf16 = mybir.dt.bfloat16
    split_last = variant in ("prow3", "prow23")
    scalar_cast = variant in ("prow2", "prow23")
    nc = bacc.Bacc("TRN2", target_bir_lowering=False, debug=False)

    # ---- DRAM inputs (host pre-packed layouts) --------------------------
    # wv_j[p, k, c] = bf16(Wv[k*128 + p, j*128 + c])  (column block j)
    n_full = KC - 2 if split_last else KC
    wvs_d = {
        j: nc.dram_tensor(f"wv{j}", [128, KC, 128], bf16, kind="ExternalInput")
        for j in range(n_full)
    }
    wvh_d = {}
    if split_last:
        for j in (KC - 2, KC - 1):
            for h in ("a", "b"):
                wvh_d[(j, h)] = nc.dram_tensor(
                    f"wv{j}{h}", [128, KC // 2, 128], bf16, kind="ExternalInput"
                )
    # vft[p, k*4 + b] = vf[b, k*128 + p]
    vft_d = nc.dram_tensor("vft", [128, 32], bf16, kind="ExternalInput")
    # wp halves: wpa[p, j, c] = Wp[j*128 + p, ci_c] for j in 0..3; wpb j 4..7
    wpa_d = nc.dram_tensor("wpa", [128, 4, CSH], bf16, kind="ExternalInput")
    wpb_d = nc.dram_tensor("wpb", [128, 4, CSH], bf16, kind="ExternalInput")
    # hdr2 row 0:
    #   [0, 0:1024]      bv
    #   [0:4, 1024:1536] mask; [0, 1668:2180] ones512
    #   [0, 1536:1664]   bp[ci]
    #   [0, 1664:1668]   ones4
    hdr2_d = nc.dram_tensor("hdr2", [5, 2180], bf16, kind="ExternalInput")
    # out[p, q, tc, b] = out_full[b, q*128 + tc, ci_p]  (1KB runs per q)
    out = nc.dram_tensor("out", [128, KC, 128, B], bf16, kind="ExternalOutput")

    with tile.TileContext(nc) as tc:
        with (
            tc.tile_pool(name="sb", bufs=1) as sb,
            tc.tile_pool(name="pv", bufs=1, space="PSUM") as pv,
            tc.tile_pool(name="pb", bufs=1, space="PSUM") as pb,
        ):
            # ---- SBUF tiles -------------------------------------------------
            wvs_t = {
                j: sb.tile([128, KC, 128], bf16, name=f"wv{j}", tag=f"wv{j}")
                for j in range(n_full)
            }
            wvh_t = {
                key: sb.tile(
                    [128, KC // 2, 128], bf16,
                    name=f"wv{key[0]}{key[1]}", tag=f"wv{key[0]}{key[1]}",
                )
                for key in wvh_d
            }
            vft_t = sb.tile([128, 32], bf16, tag="vft")
            wpa_t = sb.tile([128, 4, CSH], bf16, tag="wpa")
            wpb_t = sb.tile([128, 4, CSH], bf16, tag="wpb")
            hdr2_t = sb.tile([5, 2180], bf16, tag="hdr2")
            vvt_sb = [
                sb.tile([128, B], bf16, name=f"vvt{j}", tag=f"vvt{j}")
                for j in range(KC)
            ]
            bc_t = sb.tile([128, 128 * B], bf16, tag="bc")

            vft = vft_t[:].rearrange("p (k b) -> p k b", b=B)
            bv_row = hdr2_t[0:1, 0:1024]
            mask4 = hdr2_t[0:4, 1024:1536]
            ones512 = hdr2_t[0:1, 1668:2180]
            bp_row = hdr2_t[0:1, 1536:1664]
            ones4 = hdr2_t[0:1, 1664:1668]

            def wv_block(j):
                return wvs_t[j][:] if j in wvs_t else None

            # ---- PSUM tiles -------------------------------------------------
            psum_vvt = [
                pv.tile([128, B], f32, name=f"pvt{h}", tag=f"pvt{h}")
                for h in range(3)
            ]
            # (bf16 PSUM for the mask matmul was tried and is rejected by
            # bass: matmul output must be fp32; only transpose-mode writes
            # bf16 PSUM.)
            psum_bc = pb.tile([128, 128 * B], f32, tag="pb")
            psum_row = (
                pv.tile([B, CSH], f32, name="psum_row", tag="pr")
                if variant.startswith("prow")
                else None
            )
            psum_dummy = (
                pv.tile([B, CSH], f32, name="psum_dummy", tag="pd")
                if variant == "proww"
                else None
            )

            # ---- DMA in (balanced dual HWDGE queues) ------------------------
            # sync:   hdr2, wv0, wpa, wv2, wv4, wv6   (1.17 MB)
            # scalar: vft, wv1, wpb, wv3, wv5, wv7    (1.16 MB)
            nc.sync.dma_start(hdr2_t[:], hdr2_d[:, :])
            nc.scalar.dma_start(vft_t[:], vft_d[:, :])
            nc.sync.dma_start(wvs_t[0][:], wvs_d[0][:, :, :])
            nc.scalar.dma_start(wvs_t[1][:], wvs_d[1][:, :, :])
            nc.sync.dma_start(wpa_t[:], wpa_d[:, :, :])
            nc.scalar.dma_start(wpb_t[:], wpb_d[:, :, :])
            for j in range(2, n_full):
                eng = nc.sync if j % 2 == 0 else nc.scalar
                eng.dma_start(wvs_t[j][:], wvs_d[j][:, :, :])
            if split_last:
                for j in (KC - 2, KC - 1):
                    nc.sync.dma_start(
                        wvh_t[(j, "a")][:], wvh_d[(j, "a")][:, :, :]
                    )
                    nc.scalar.dma_start(
                        wvh_t[(j, "b")][:], wvh_d[(j, "b")][:, :, :]
                    )

            if variant == "fold":
                # fold-broadcast opener: psum_bc[ci, (tc, b)] += bp[ci]
                nc.tensor.matmul(
                    psum_bc[:], bp_row, ones512, start=True, stop=False
                )

            # ---- per column-block: vvT_j, then fold --------------------------
            for j in range(KC):
                pt = psum_vvt[j % 3]
                blk = wv_block(j)
                # K=1 bias opener: vvT_j[c', b] = bv[j*128 + c']
                nc.tensor.matmul(
                    pt[:],
                    bv_row[:, j * 128 : (j + 1) * 128],
                    ones4,
                    start=True,
                    stop=False,
                )
                for k in range(KC):
                    if split_last and j >= KC - 2:
                        half = "a" if k < KC // 2 else "b"
                        wsrc = wvh_t[(j, half)][:, k % (KC // 2), :]
                    else:
                        wsrc = blk[:, k, :]
                    nc.tensor.matmul(
                        pt[:],
                        wsrc,
                        vft[:, k, :],
                        start=False,
                        stop=(k == KC - 1),
                    )
                nc.vector.tensor_copy(vvt_sb[j][:], pt[:])
                wp_half = wpa_t if j < 4 else wpb_t
                if variant == "fold":
                    # += sum_{c' in block j} Wp[c', ci] * vvT[c', b]  (all tc)
                    va = vvt_sb[j][:]
                    vrep = bass.AP(
                        va.tensor,
                        va.offset,
                        [list(va.ap[0]), [0, 128], list(va.ap[1])],
                    )
                    nc.tensor.matmul(
                        psum_bc[:],
                        wp_half[:, j % 4, :],
                        vrep,
                        start=False,
                        stop=(j == KC - 1),
                    )
                else:
                    if j == 0:
                        nc.tensor.matmul(
                            psum_row[:], ones4, bp_row, start=True, stop=False
                        )
                    nc.tensor.matmul(
                        psum_row[:],
                        vvt_sb[j][:],
                        wp_half[:, j % 4, :],
                        start=False,
                        stop=(j == KC - 1),
                    )
                    if variant == "proww" and 1 <= j < KC - 1:
                        # HAM warm-keepers: junk matmuls on early-arrived
                        # data fill the PE idle gaps between sem-gated
                        # chains so the tail runs at the 2.4 GHz rate.
                        for _d in range(3):
                            nc.tensor.matmul(
                                psum_dummy[:],
                                vft[:, 0, :],
                                wvs_t[0][:, _d, :],
                                start=True,
                                stop=True,
                            )

            if variant.startswith("prow"):
                prow_sb = sb.tile([B, CSH], bf16, name="prow_sb", tag="prow")
                nc.vector.tensor_copy(prow_sb[:], psum_row[:])
                nc.tensor.matmul(
                    psum_bc[:], prow_sb[:], mask4, start=True, stop=True
                )
            if scalar_cast:
                nc.vector.tensor_copy(bc_t[:, 0:256], psum_bc[:, 0:256])
                nc.scalar.copy(bc_t[:, 256:512], psum_bc[:, 256:512])
            else:
                nc.vector.tensor_copy(bc_t[:], psum_bc[:])

            # ---- out DMA: replicated source over q ---------------------------
            out_v = out.rearrange("p q t b -> p q (t b)")
            bca = bc_t[:]
            if variant == "prow1":
                rep = bass.AP(
                    bca.tensor,
                    bca.offset,
                    [list(bca.ap[0]), [0, KC], list(bca.ap[1])],
                )
                nc.sync.dma_start(out_v[:, :, :], rep)
            else:
                rep = bass.AP(
                    bca.tensor,
                    bca.offset,
                    [list(bca.ap[0]), [0, KC // 2], list(bca.ap[1])],
                )
                nc.sync.dma_start(out_v[:, 0 : KC // 2, :], rep)
                nc.scalar.dma_start(out_v[:, KC // 2 : KC, :], rep)

    nc.compile()
    return nc


def _get_built(variant=None):
    variant = variant or VARIANT
    if variant not in _BUILT:
        _BUILT[variant] = build_nc(variant)
    return _BUILT[variant]


def make_in_maps(inputs, variant=None):
    variant = variant or VARIANT
    split_last = variant in ("prow3", "prow23")
    import ml_dtypes

    bf16 = ml_dtypes.bfloat16

    vf = np.asarray(inputs["visual_features"], np.float32)
    wv = np.asarray(inputs["Wv"], np.float32)
    wp = np.asarray(inputs["Wp"], np.float32)
    bv = np.asarray(inputs["bv"], np.float32)
    bp = np.asarray(inputs["bp"], np.float32)

    # vfT chunks: [p, k*4 + b] = vf[b, k*128 + p]
    vft_np = np.ascontiguousarray(
        vf.T.reshape(KC, 128, B).transpose(1, 0, 2).reshape(128, KC * B)
    ).astype(bf16)
    # wv column blocks: wv_j[p, k, c] = Wv[k*128 + p, j*128 + c]
    wv_bf = wv.astype(bf16)

    def blockp(j):
        return (
            wv_bf[:, j * 128 : (j + 1) * 128].reshape(KC, 128, 128).transpose(1, 0, 2)
        )

    wv_singles = {j: np.ascontiguousarray(blockp(j)) for j in range(KC)}

    # hdr2 shared part: bv + mask (rows 0-3) + ones512 (row 4) + ones4
    hdr2_base = np.zeros((5, 2180), np.float32)
    hdr2_base[0, 0:1024] = bv
    for b in range(B):
        hdr2_base[b, 1024 + b : 1536 : B] = 1.0  # mask[b, tc*4 + b] = 1
    hdr2_base[0, 1668:2180] = 1.0
    hdr2_base[0, 1664:1668] = 1.0

    maps = []
    for i in range(N_CORES):
        ci = slice(i * CSH, (i + 1) * CSH)
        # wp_p[p, j, c] = Wp[j*128 + p, ci_c]
        wp_p = wp[:, ci].reshape(KC, 128, CSH).transpose(1, 0, 2).astype(bf16)
        hdr2 = hdr2_base.copy()
        hdr2[0, 1536:1664] = bp[ci]
        m = {
            "vft": vft_np,
            "wpa": np.ascontiguousarray(wp_p[:, 0:4, :]),
            "wpb": np.ascontiguousarray(wp_p[:, 4:8, :]),
            "hdr2": hdr2.astype(bf16),
        }
        n_full = KC - 2 if split_last else KC
        for j in range(n_full):
            m[f"wv{j}"] = wv_singles[j]
        if split_last:
            for j in (KC - 2, KC - 1):
                m[f"wv{j}a"] = np.ascontiguousarray(wv_singles[j][:, 0 : KC // 2, :])
                m[f"wv{j}b"] = np.ascontiguousarray(wv_singles[j][:, KC // 2 :, :])
        maps.append(m)
    return maps


def run(inputs, trace=False, variant=None, **kw):
    from concourse.bass_utils import run_bass_kernel_spmd

    nc = _get_built(variant)
    res = run_bass_kernel_spmd(
        nc,
        make_in_maps(inputs, variant or VARIANT),
        core_ids=list(range(N_CORES)),
        trace=trace,
        **kw,
    )
    full = np.empty((B, T, C), np.float32)
    for i, r in enumerate(res.results):
        # out[p, q, tc, b] -> full[b, q*128 + tc, ci_p]
        o = np.asarray(r["out"]).astype(np.float32)
        full[:, :, i * CSH : (i + 1) * CSH] = o.transpose(3, 1, 2, 0).reshape(
            B, T, CSH
        )
    return full, res


def kernel(**inputs) -> np.ndarray:
    full, _ = run(inputs, trace=False)
    return full


# revision 51
# speedup vs baseline: 1.0289x; 1.0289x over previous
"""Trainium2 Bass kernel for nn_CrossAttention_47502338294587.

Math: the reference cross-attention has a single KV position broadcast over
all T query positions.  Softmax over a row of identical logits is uniform,
so attention output == v for every query, and the whole module collapses to

    out[b, t, :] = (visual_features[b] @ Wv + bv) @ Wp + bp      (for all t)

independent of x / Wq / Wk.  The device computes the two projections and
broadcasts the per-batch row over the T axis; the host only does input
layout prep (incl. bf16 weight packing) and shard re-assembly.

Sharding: tensor-parallel over the output channel dim C - core i computes
and writes out[:, :, i*128:(i+1)*128] (full Wv, column shard of Wp / bp).

Pipeline (per core):
  Wv streams as 8 column blocks wv_j = Wv[:, j*128:(j+1)*128] packed
  [p, k, c] = Wv[k*128+p, j*128+c], 256KB each, balanced across both
  HWDGE queues (12 input DMAs; the 4 tiny const transfers complete
  early so DMA-sem reuse never stalls an issue).  As each block lands
  the PE computes the transposed projection directly:

    psum_vvT_j[c', b] = bv_j (K=1 opener) + sum_k wv_j[:,k,:]^T @ vfT_k

  (no 4-partition PSUM casts, no PE transposes).  A [128,4] DVE copy
  drops vvT_j to SBUF and one accumulating matmul folds it into the
  output row while later blocks still stream (variant "prow", default):

    psum_row[b, ci] = bp (K=1 opener) + sum_j vvT_j^T @ Wp_j

  The broadcast uses a constant one-hot mask (no DVE multiply):

    pbcT[ci, (tc, b')] = sum_b prow[b, ci] * mask[b, (tc, b')],
    mask[b, (tc,b')] = (b == b')

  Variant "fold" instead folds each block straight into the broadcast
  PSUM tile with a tc-stride-0 replicated rhs (no prow/mask stage);
  "prow1" is prow with a single-queue out DMA.  All three are within
  run-to-run noise of each other; prow matched the best measurements.

  The resulting [128, 512] bf16 tile is replicated by the out DMA over
  the 8 T-chunks, split across both HWDGE queues.  Out layout is
  out[p=ci_local, q, tc, b] (1KB contiguous runs); the host transposes
  and upcasts to f32 during re-assembly.
"""

import os
import sys

import numpy as np

for _p in ("/opt/trn_rl_repo",):
    if _p not in sys.path and os.path.isdir(_p):
        sys.path.insert(0, _p)

B, T, C = 4, 1024, 1024
N_CORES = 8
CSH = C // N_CORES  # 128, C-shard per core
KC = C // 128  # 8 contraction chunks

_BUILT = {}
VARIANT = os.environ.get("KERNEL_VARIANT", "prow")


def build_nc(variant="fold"):
    """Build + compile the Bass program (one NeuronCore's SPMD body).

    variant "fold": per-block matmul folds directly into the broadcast
    PSUM tile via a tc-replicated rhs (short tail).
    variant "prow": per-block matmul accumulates prow[b, ci] (N=128),
    then a one-hot-mask matmul broadcasts it (v2/v4 tail).
    variant "prow1": prow tail with a single-queue out DMA.
    """
    import concourse.bass as bass
    import concourse.mybir as mybir
    import concourse.tile as tile
    from concourse import bacc

    f32 = mybir.dt.float32
    bf16 = mybir.dt.bfloat16
    split_last = variant in ("prow3", "prow23")
    scalar_cast = variant in ("prow2", "prow23")
    nc = bacc.Bacc("TRN2", target_bir_lowering=False, debug=False)

    # ---- DRAM inputs (host pre-packed layouts) --------------------------
    # wv_j[p, k, c] = bf16(Wv[k*128 + p, j*128 + c])  (column block j)
    n_full = KC - 2 if split_last else KC
    wvs_d = {
        j: nc.dram_tensor(f"wv{j}", [128, KC, 128], bf16, kind="ExternalInput")
        for j in range(n_full)
    }
    wvh_d = {}
    if split_last:
        for j in (KC - 2, KC - 1):
            for h in ("a", "b"):
                wvh_d[(j, h)] = nc.dram_tensor(
                    f"wv{j}{h}", [128, KC // 2, 128], bf16, kind="ExternalInput"
                )
    # vft[p, k*4 + b] = vf[b, k*128 + p]
    vft_d = nc.dram_tensor("vft", [128, 32], bf16, kind="ExternalInput")
    # wp halves: wpa[p, j, c] = Wp[j*128 + p, ci_c] for j in 0..3; wpb j 4..7
    wpa_d = nc.dram_tensor("wpa", [128, 4, CSH], bf16, kind="ExternalInput")
    wpb_d = nc.dram_tensor("wpb", [128, 4, CSH], bf16, kind="ExternalInput")
    # hdr2 row 0:
    #   [0, 0:1024]      bv
    #   [0:4, 1024:1536] mask; [0, 1668:2180] ones512
    #   [0, 1536:1664]   bp[ci]
    #   [0, 1664:1668]   ones4
    hdr2_d = nc.dram_tensor("hdr2", [5, 2180], bf16, kind="ExternalInput")
    # out[p, q, tc, b] = out_full[b, q*128 + tc, ci_p]  (1KB runs per q)
    out = nc.dram_tensor("out", [128, KC, 128, B], bf16, kind="ExternalOutput")

    with tile.TileContext(nc) as tc:
        with (
            tc.tile_pool(name="sb", bufs=1) as sb,
            tc.tile_pool(name="pv", bufs=1, space="PSUM") as pv,
            tc.tile_pool(name="pb", bufs=1, space="PSUM") as pb,
        ):
            # ---- SBUF tiles -------------------------------------------------
            wvs_t = {
                j: sb.tile([128, KC, 128], bf16, name=f"wv{j}", tag=f"wv{j}")
                for j in range(n_full)
            }
            wvh_t = {
                key: sb.tile(
                    [128, KC // 2, 128], bf16,
                    name=f"wv{key[0]}{key[1]}", tag=f"wv{key[0]}{key[1]}",
                )
                for key in wvh_d
            }
            vft_t = sb.tile([128, 32], bf16, tag="vft")
            wpa_t = sb.tile([128, 4, CSH], bf16, tag="wpa")
            wpb_t = sb.tile([128, 4, CSH], bf16, tag="wpb")
            hdr2_t = sb.tile([5, 2180], bf16, tag="hdr2")
            vvt_sb = [
                sb.tile([128, B], bf16, name=f"vvt{j}", tag=f"vvt{j}")
                for j in range(KC)
            ]
            bc_t = sb.tile([128, 128 * B], bf16, tag="bc")

            vft = vft_t[:].rearrange("p (k b) -> p k b", b=B)
            bv_row = hdr2_t[0:1, 0:1024]
            mask4 = hdr2_t[0:4, 1024:1536]
            ones512 = hdr2_t[0:1, 1668:2180]
            bp_row = hdr2_t[0:1, 1536:1664]
            ones4 = hdr2_t[0:1, 1664:1668]

            def wv_block(j):
                return wvs_t[j][:] if j in wvs_t else None

            # ---- PSUM tiles -------------------------------------------------
            psum_vvt = [
                pv.tile([128, B], f32, name=f"pvt{h}", tag=f"pvt{h}")
                for h in range(3)
            ]
            # (bf16 PSUM for the mask matmul was tried and is rejected by
            # bass: matmul output must be fp32; only transpose-mode writes
            # bf16 PSUM.)
            psum_bc = pb.tile([128, 128 * B], f32, tag="pb")
            psum_row = (
                pv.tile([B, CSH], f32, name="psum_row", tag="pr")
                if variant.startswith("prow")
                else None
            )
            psum_dummy = (
                pv.tile([B, CSH], f32, name="psum_dummy", tag="pd")
                if variant == "proww"
                else None
            )

            # ---- DMA in (balanced dual HWDGE queues) ------------------------
            # sync:   hdr2, wv0, wpa, wv2, wv4, wv6   (1.17 MB)
            # scalar: vft, wv1, wpb, wv3, wv5, wv7    (1.16 MB)
            nc.sync.dma_start(hdr2_t[:], hdr2_d[:, :])
            nc.scalar.dma_start(vft_t[:], vft_d[:, :])
            nc.sync.dma_start(wvs_t[0][:], wvs_d[0][:, :, :])
            nc.scalar.dma_start(wvs_t[1][:], wvs_d[1][:, :, :])
            nc.sync.dma_start(wpa_t[:], wpa_d[:, :, :])
            nc.scalar.dma_start(wpb_t[:], wpb_d[:, :, :])
            for j in range(2, n_full):
                eng = nc.sync if j % 2 == 0 else nc.scalar
                eng.dma_start(wvs_t[j][:], wvs_d[j][:, :, :])
            if split_last:
                for j in (KC - 2, KC - 1):
                    nc.sync.dma_start(
                        wvh_t[(j, "a")][:], wvh_d[(j, "a")][:, :, :]
                    )
                    nc.scalar.dma_start(
                        wvh_t[(j, "b")][:], wvh_d[(j, "b")][:, :, :]
                    )

            if variant == "fold":
                # fold-broadcast opener: psum_bc[ci, (tc, b)] += bp[ci]
                nc.tensor.matmul(
                    psum_bc[:], bp_row, ones512, start=True, stop=False
                )

            # ---- per column-block: vvT_j, then fold --------------------------
            for j in range(KC):
                pt = psum_vvt[j % 3]
                blk = wv_block(j)
                # K=1 bias opener: vvT_j[c', b] = bv[j*128 + c']
                nc.tensor.matmul(
                    pt[:],
                    bv_row[:, j * 128 : (j + 1) * 128],
                    ones4,
                    start=True,
                    stop=False,
                )
                for k in range(KC):
                    if split_last and j >= KC - 2:
                        half = "a" if k < KC // 2 else "b"
                        wsrc = wvh_t[(j, half)][:, k % (KC // 2), :]
                    else:
                        wsrc = blk[:, k, :]
                    nc.tensor.matmul(
                        pt[:],
                        wsrc,
                        vft[:, k, :],
                        start=False,
                        stop=(k == KC - 1),
                    )
                nc.vector.tensor_copy(vvt_sb[j][:], pt[:])
                wp_half = wpa_t if j < 4 else wpb_t
                if variant == "fold" or (variant == "prowh" and j == KC - 1):
                    # += sum_{c' in block j} Wp[c', ci] * vvT[c', b]  (all tc)
                    va = vvt_sb[j][:]
                    vrep = bass.AP(
                        va.tensor,
                        va.offset,
                        [list(va.ap[0]), [0, 128], list(va.ap[1])],
                    )
                    nc.tensor.matmul(
                        psum_bc[:],
                        wp_half[:, j % 4, :],
                        vrep,
                        start=False,
                        stop=(j == KC - 1),
                    )
                else:
                    if j == 0:
                        nc.tensor.matmul(
                            psum_row[:], ones4, bp_row, start=True, stop=False
                        )
                    nc.tensor.matmul(
                        psum_row[:],
                        vvt_sb[j][:],
                        wp_half[:, j % 4, :],
                        start=False,
                        stop=(j == KC - 1 - (variant == "prowh")),
                    )
                    if variant == "prowh" and j == KC - 2:
                        # broadcast blocks 0..6 (+bp) into psum_bc now, in
                        # the PE idle gap while block 7's DMA completes;
                        # block 7 then folds directly into psum_bc above.
                        prowh_sb = sb.tile(
                            [B, CSH], bf16, name="prowh_sb", tag="prowh"
                        )
                        nc.vector.tensor_copy(prowh_sb[:], psum_row[:])
                        nc.tensor.matmul(
                            psum_bc[:], prowh_sb[:], mask4,
                            start=True, stop=False,
                        )
                    if variant == "proww" and 1 <= j < KC - 1:
                        # HAM warm-keepers: junk matmuls on early-arrived
                        # data fill the PE idle gaps between sem-gated
                        # chains so the tail runs at the 2.4 GHz rate.
                        for _d in range(3):
                            nc.tensor.matmul(
                                psum_dummy[:],
                                vft[:, 0, :],
                                wvs_t[0][:, _d, :],
                                start=True,
                                stop=True,
                            )

            if variant.startswith("prow") and variant != "prowh":
                prow_sb = sb.tile([B, CSH], bf16, name="prow_sb", tag="prow")
                nc.vector.tensor_copy(prow_sb[:], psum_row[:])
                nc.tensor.matmul(
                    psum_bc[:], prow_sb[:], mask4, start=True, stop=True
                )
            if scalar_cast:
                nc.vector.tensor_copy(bc_t[:, 0:256], psum_bc[:, 0:256])
                nc.scalar.copy(bc_t[:, 256:512], psum_bc[:, 256:512])
            else:
                nc.vector.tensor_copy(bc_t[:], psum_bc[:])

            # ---- out DMA: replicated source over q ---------------------------
            out_v = out.rearrange("p q t b -> p q (t b)")
            bca = bc_t[:]
            if variant == "prow1":
                rep = bass.AP(
                    bca.tensor,
                    bca.offset,
                    [list(bca.ap[0]), [0, KC], list(bca.ap[1])],
                )
                nc.sync.dma_start(out_v[:, :, :], rep)
            else:
                rep = bass.AP(
                    bca.tensor,
                    bca.offset,
                    [list(bca.ap[0]), [0, KC // 2], list(bca.ap[1])],
                )
                nc.sync.dma_start(out_v[:, 0 : KC // 2, :], rep)
                nc.scalar.dma_start(out_v[:, KC // 2 : KC, :], rep)

    nc.compile()
    return nc


def _get_built(variant=None):
    variant = variant or VARIANT
    if variant not in _BUILT:
        _BUILT[variant] = build_nc(variant)
    return _BUILT[variant]


def make_in_maps(inputs, variant=None):
    variant = variant or VARIANT
    split_last = variant in ("prow3", "prow23")
    import ml_dtypes

    bf16 = ml_dtypes.bfloat16

    vf = np.asarray(inputs["visual_features"], np.float32)
    wv = np.asarray(inputs["Wv"], np.float32)
    wp = np.asarray(inputs["Wp"], np.float32)
    bv = np.asarray(inputs["bv"], np.float32)
    bp = np.asarray(inputs["bp"], np.float32)

    # vfT chunks: [p, k*4 + b] = vf[b, k*128 + p]
    vft_np = np.ascontiguousarray(
        vf.T.reshape(KC, 128, B).transpose(1, 0, 2).reshape(128, KC * B)
    ).astype(bf16)
    # wv column blocks: wv_j[p, k, c] = Wv[k*128 + p, j*128 + c]
    wv_bf = wv.astype(bf16)

    def blockp(j):
        return (
            wv_bf[:, j * 128 : (j + 1) * 128].reshape(KC, 128, 128).transpose(1, 0, 2)
        )

    wv_singles = {j: np.ascontiguousarray(blockp(j)) for j in range(KC)}

    # hdr2 shared part: bv + mask (rows 0-3) + ones512 (row 4) + ones4
    hdr2_base = np.zeros((5, 2180), np.float32)
    hdr2_base[0, 0:1024] = bv
    for b in range(B):
        hdr2_base[b, 1024 + b : 1536 : B] = 1.0  # mask[b, tc*4 + b] = 1
    hdr2_base[0, 1668:2180] = 1.0
    hdr2_base[0, 1664:1668] = 1.0

    maps = []
    for i in range(N_CORES):
        ci = slice(i * CSH, (i + 1) * CSH)
        # wp_p[p, j, c] = Wp[j*128 + p, ci_c]
        wp_p = wp[:, ci].reshape(KC, 128, CSH).transpose(1, 0, 2).astype(bf16)
        hdr2 = hdr2_base.copy()
        hdr2[0, 1536:1664] = bp[ci]
        m = {
            "vft": vft_np,
            "wpa": np.ascontiguousarray(wp_p[:, 0:4, :]),
            "wpb": np.ascontiguousarray(wp_p[:, 4:8, :]),
            "hdr2": hdr2.astype(bf16),
        }
        n_full = KC - 2 if split_last else KC
        for j in range(n_full):
            m[f"wv{j}"] = wv_singles[j]
        if split_last:
            for j in (KC - 2, KC - 1):
                m[f"wv{j}a"] = np.ascontiguousarray(wv_singles[j][:, 0 : KC // 2, :])
                m[f"wv{j}b"] = np.ascontiguousarray(wv_singles[j][:, KC // 2 :, :])
        maps.append(m)
    return maps


def run(inputs, trace=False, variant=None, **kw):
    from concourse.bass_utils import run_bass_kernel_spmd

    nc = _get_built(variant)
    res = run_bass_kernel_spmd(
        nc,
        make_in_maps(inputs, variant or VARIANT),
        core_ids=list(range(N_CORES)),
        trace=trace,
        **kw,
    )
    full = np.empty((B, T, C), np.float32)
    for i, r in enumerate(res.results):
        # out[p, q, tc, b] -> full[b, q*128 + tc, ci_p]
        o = np.asarray(r["out"]).astype(np.float32)
        full[:, :, i * CSH : (i + 1) * CSH] = o.transpose(3, 1, 2, 0).reshape(
            B, T, CSH
        )
    return full, res


def kernel(**inputs) -> np.ndarray:
    full, _ = run(inputs, trace=False)
    return full
